# revision 19
# baseline (speedup 1.0000x reference)
"""BiDAF attention kernel for Trainium2 (8 NeuronCores, data-parallel over batch).

Problem (per full input): B=16, L=M=1024, H=128
  s  = text@tw + (mod@mw).T + (text*tmw)@mod.T + bias          (B, L, M)
  p1 = softmax_M(mmask*s + (1-mmask)*NEG)
  p2 = softmax_L(tmask*s + (1-tmask)*NEG)
  a  = p1 @ mod
  b  = p1 @ p2.T @ text        (computed as p1 @ (p2.T @ text))
  out = [text, a, text*a, text*b]                               (B, L, 4H)

Key facts used:
  * softmax_M is invariant to per-row (per-l) shifts: s0 & bias drop from p1.
  * softmax_L is invariant to per-column (per-m) shifts: s1 & bias drop from p2.
  * masking with {0,1} is equivalent to adding (mask-1)*30000 before exp.
  * a ones-column appended to the rhs of the p1/p2 contraction matmuls
    yields the softmax denominators for free (an extra output column).
  * fp32 matmuls run 2-pass (LOW_HIGH) on trn2 — all matmul operands are
    kept in bf16 (PSUM accumulation and softmax normalization stay fp32).
  * sparsity: masked m contribute exactly 0 to p1 (and masked l to p2), so
    the m- and l-spaces are compacted to the unmasked rows. The host
    computes permutation indices from the masks (metadata); the device
    gathers the rows via indirect DMA and computes only ceil(Mu/128) /
    ceil(Lu/128) chunks. Output rows (all l) are never compacted.

Each of the 8 cores processes 2 batch items; no cross-core communication.
"""

import numpy as np

B, L, M, H = 16, 1024, 1024, 128
NCORES = 8
BPC = B // NCORES  # batches per core
P = 128
LT, MT = L // P, M // P
NEGB = 30000.0

_CACHE = {}


def _build(MU, LU):
    """Builds the per-core Bass program for MU gathered m-chunks and LU
    gathered l-chunks (SPMD: same NEFF on all 8 cores)."""
    from contextlib import ExitStack

    import concourse.bass as bass
    import concourse.mybir as mybir
    import concourse.tile as tile
    from concourse import bacc
    from concourse.bass import ts
    from concourse.masks import make_identity

    f32 = mybir.dt.float32
    bf16 = mybir.dt.bfloat16
    i32 = mybir.dt.int32
    Exp = mybir.ActivationFunctionType.Exp
    Alu = mybir.AluOpType

    nc = bacc.Bacc(name="bidaf8")
    text = nc.dram_tensor("text", (BPC, L, H), f32, kind="ExternalInput").ap()
    # gathered-space metadata (host-computed from the masks):
    #   lidx/midx: [p, c] = flattened row index (b*L + perm[c*128+p])
    #   tmg/mmg:   [p, c] = mask value at that gathered position (0/1)
    textg = nc.dram_tensor("text_g", (BPC, P, LU, H), f32,
                           kind="ExternalInput").ap()
    modg = nc.dram_tensor("mod_g", (BPC, P, MU, H), f32,
                          kind="ExternalInput").ap()
    tmg = nc.dram_tensor("tmask_g", (BPC, P, LU), i32, kind="ExternalInput").ap()
    mmg = nc.dram_tensor("mmask_g", (BPC, P, MU), i32, kind="ExternalInput").ap()
    wt = nc.dram_tensor("w_text", (H, 1), f32, kind="ExternalInput").ap()
    wm = nc.dram_tensor("w_mod", (H, 1), f32, kind="ExternalInput").ap()
    wtm = nc.dram_tensor("w_tm", (H, 1), f32, kind="ExternalInput").ap()
    out = nc.dram_tensor("out", (BPC, L, 4 * H), f32, kind="ExternalOutput").ap()

    MG = MU * P  # gathered m columns
    NE2 = [min(512, MG - i * 512) for i in range((MG + 511) // 512)]

    def rep_rows(col_ap):
        # (H, 1) DRAM column -> broadcast AP read as (P, H): every partition
        # reads the same H contiguous floats. (gpsimd DMA only)
        return bass.AP(tensor=col_ap.tensor, offset=col_ap.offset,
                       ap=[[0, P], col_ap.ap[0]])

    with tile.TileContext(nc) as tc, ExitStack() as ctx:
        const = ctx.enter_context(tc.tile_pool(name="const", bufs=1))
        oper = ctx.enter_context(tc.tile_pool(name="oper", bufs=2))
        big = ctx.enter_context(tc.tile_pool(name="big", bufs=2))
        small = ctx.enter_context(tc.tile_pool(name="small", bufs=2))
        outp = ctx.enter_context(tc.tile_pool(name="outp", bufs=4))
        ps_s = ctx.enter_context(tc.tile_pool(name="ps_s", bufs=3, space="PSUM"))
        ps_q = ctx.enter_context(tc.tile_pool(name="ps_q", bufs=5, space="PSUM"))

        ident16 = const.tile([P, P], bf16)
        make_identity(nc, ident16)
        wtm_sb = const.tile([P, 1], f32)
        nc.sync.dma_start(wtm_sb, wtm)
        wt_rep = const.tile([P, H], f32)
        nc.gpsimd.dma_start(wt_rep, rep_rows(wt))
        wm_rep = const.tile([P, H], f32)
        nc.gpsimd.dma_start(wm_rep, rep_rows(wm))

        st = []  # per-batch tiles
        for b in range(BPC):
            d = {}
            st.append(d)
            # ---- gathered masks -> bias partials ----
            tmgi = small.tile([P, LU], i32, tag="tmgi")
            nc.gpsimd.dma_start(tmgi, tmg[b])
            d["bias2"] = small.tile([P, LU], f32, tag="bias2", name="bias2")  # per gathered l
            tmgf = small.tile([P, LU], f32, tag="tmgf")
            nc.vector.tensor_copy(tmgf, tmgi)
            nc.vector.tensor_scalar(d["bias2"], tmgf, 1.0, NEGB,
                                    op0=Alu.subtract, op1=Alu.mult)
            mmgi = small.tile([P, MU], i32, tag="mmgi")
            nc.gpsimd.dma_start(mmgi, mmg[b])
            d["bias1"] = small.tile([P, MU], f32, tag="bias1", name="bias1")  # per gathered m
            mmgf = small.tile([P, MU], f32, tag="mmgf")
            nc.vector.tensor_copy(mmgf, mmgi)
            nc.vector.tensor_scalar(d["bias1"], mmgf, 1.0, NEGB,
                                    op0=Alu.subtract, op1=Alu.mult)

            # ---- natural text load + host-gathered row loads ----
            d["txt"] = oper.tile([P, LT, H], f32, tag="txt", name="txt")
            nc.sync.dma_start(d["txt"],
                              text[b].rearrange("(p o) h -> p o h", p=P))
            txtg = oper.tile([P, LU, H], f32, tag="txtg")
            nc.scalar.dma_start(txtg, textg[b])
            modsg = oper.tile([P, MU, H], f32, tag="modsg")
            nc.gpsimd.dma_start(modsg, modg[b])

            # ---- bf16 casts ----
            d["txt16"] = oper.tile([P, LT, H], bf16, tag="txt16", name="txt16")
            nc.vector.tensor_copy(d["txt16"], d["txt"])
            d["txtg16"] = oper.tile([P, LU, H + 1], bf16, tag="txtg16", name="txtg16")
            nc.vector.memset(d["txtg16"][:, :, H : H + 1], 1.0)
            nc.vector.tensor_copy(d["txtg16"][:, :, :H], txtg)
            d["modwq"] = big.tile([P, MU, 2 * H + 1], bf16, tag="modwq", name="modwq")
            nc.vector.memset(d["modwq"][:, :, H : H + 1], 1.0)
            nc.vector.tensor_copy(d["modwq"][:, :, :H], modsg)

            # ---- s0 (gathered l) / s1 (gathered m) row-dots on DVE ----
            s0col = small.tile([P, LU], f32, tag="s0col")
            for c in range(LU):
                scr = small.tile([P, H], f32, tag="scr")
                nc.vector.scalar_tensor_tensor(
                    out=scr, in0=txtg[:, c, :], scalar=1.0, in1=wt_rep,
                    op0=Alu.mult, op1=Alu.mult,
                    accum_out=s0col[:, c : c + 1])
            nc.vector.tensor_add(d["bias2"], d["bias2"], s0col)
            s1col = small.tile([P, MU], f32, tag="s1col")
            for c in range(MU):
                scr = small.tile([P, H], f32, tag="scr")
                nc.vector.scalar_tensor_tensor(
                    out=scr, in0=modsg[:, c, :], scalar=1.0, in1=wm_rep,
                    op0=Alu.mult, op1=Alu.mult,
                    accum_out=s1col[:, c : c + 1])
            nc.vector.tensor_add(d["bias1"], d["bias1"], s1col)

        for b in range(BPC):
            d = st[b]
            txt, txt16, txtg16 = d["txt"], d["txt16"], d["txtg16"]
            modwq, bias1, bias2 = d["modwq"], d["bias1"], d["bias2"]

            # ---- transposes (bf16) ----
            # txtT: (H, L) all l (rhs of E1T matmul); XgT: (H, LU*128)
            # gathered l, scaled by w_tm (lhsT of E2 matmul);
            # modTg: (H, MU*128) gathered m
            txtT = oper.tile([P, L], bf16, tag="txtT")
            for j in range(LT):
                tp = ps_q.tile([P, P], bf16, tag="q")
                nc.tensor.transpose(tp, txt16[:, j, :], ident16)
                nc.vector.tensor_copy(txtT[:, ts(j, P)], tp)
            XgT = oper.tile([P, LU * P], bf16, tag="XgT")
            for c in range(LU):
                tp = ps_q.tile([P, P], bf16, tag="q")
                nc.tensor.transpose(tp, txtg16[:, c, :H], ident16)
                nc.scalar.copy(XgT[:, ts(c, P)], tp)
            modTg = oper.tile([P, MU * P], bf16, tag="modTg")
            for c in range(MU):
                tp = ps_q.tile([P, P], bf16, tag="q")
                nc.tensor.transpose(tp, modwq[:, c, :H], ident16)
                nc.scalar.copy(modTg[:, ts(c, P)], tp)

            # scale by w_tm (per-partition h)
            nc.vector.tensor_scalar_mul(txtT, txtT, wtm_sb)
            nc.vector.tensor_scalar_mul(XgT, XgT, wtm_sb)

            # ---- E2[lg, mg] = exp(sg + bias2[lg]) ----
            E2 = big.tile([P, LU, MG], bf16, tag="E2")
            for c in range(LU):
                for hi, n in enumerate(NE2):
                    hs = slice(hi * 512, hi * 512 + n)
                    sp = ps_s.tile([P, 512], f32, tag="s")
                    nc.tensor.matmul(sp[:, :n], XgT[:, ts(c, P)], modTg[:, hs],
                                     start=True, stop=True)
                    nc.scalar.activation(E2[:, c, hs], sp[:, :n], Exp,
                                         bias=bias2[:, c : c + 1], scale=1.0)

            # ---- E1T[mg, l] = exp(sTg + bias1[mg]) interleaved with q2 ----
            E1T = big.tile([P, MU, L], bf16, tag="E1T")
            for k in range(MU):
                for half in range(2):
                    hs = ts(half, 512)
                    sp = ps_s.tile([P, 512], f32, tag="s")
                    nc.tensor.matmul(sp, modTg[:, ts(k, P)], txtT[:, hs],
                                     start=True, stop=True)
                    nc.scalar.activation(E1T[:, k, hs], sp, Exp,
                                         bias=bias1[:, k : k + 1], scale=1.0)
                # q2[mg,:] = E2.T @ [text_g|1]; wq = q2/D2
                qp = ps_q.tile([P, H + 1], f32, tag="q")
                for c in range(LU):
                    nc.tensor.matmul(qp, E2[:, c, ts(k, P)], txtg16[:, c, :],
                                     start=(c == 0), stop=(c == LU - 1))
                rec = small.tile([P, 1], f32, tag="rec2")
                nc.vector.reciprocal(rec, qp[:, H : H + 1])
                nc.scalar.mul(modwq[:, k, H + 1 :], qp[:, :H], rec)

            # ---- fused [a | D1 | b] = E1 @ [mod | 1 | wq]; assemble out ----
            for j in range(LT):
                pa = ps_q.tile([P, 2 * H + 1], f32, tag="q")
                for k in range(MU):
                    nc.tensor.matmul(pa, E1T[:, k, ts(j, P)], modwq[:, k, :],
                                     start=(k == 0), stop=(k == MU - 1))
                rec1 = small.tile([P, 1], f32, tag="rec1")
                nc.vector.reciprocal(rec1, pa[:, H : H + 1])
                o = outp.tile([P, 4 * H], f32, tag="o")
                nc.gpsimd.tensor_copy(o[:, 0:H], txt[:, j, :])
                nc.vector.tensor_scalar_mul(o[:, H : 2 * H], pa[:, :H], rec1)
                nc.vector.scalar_tensor_tensor(
                    out=o[:, 2 * H : 3 * H], in0=pa[:, :H], scalar=rec1,
                    in1=txt[:, j, :], op0=Alu.mult, op1=Alu.mult)
                nc.vector.tensor_scalar_mul(o[:, 3 * H : 4 * H],
                                            pa[:, H + 1 :], rec1)
                nc.vector.scalar_tensor_tensor(
                    out=o[:, 3 * H : 4 * H], in0=pa[:, H + 1 :], scalar=rec1,
                    in1=txt[:, j, :], op0=Alu.mult, op1=Alu.mult)
                eng = nc.sync if j % 2 == 0 else nc.gpsimd
                eng.dma_start(
                    out[b].rearrange("(p o) c -> p o c", p=P)[:, j, :], o
                )
    nc.compile()
    return nc


def get_nc(MU, LU):
    key = (MU, LU)
    if key not in _CACHE:
        _CACHE[key] = _build(MU, LU)
    return _CACHE[key]


def _gather_meta(mask, n_chunks, data):
    """mask: (N,) 0/1 int; data: (N, H). Returns (rows, mg):
    rows (P, n_chunks, H) f32 with [p, c] = data[perm[c*128+p]] and
    mg (P, n_chunks) i32 the mask at those positions, where perm lists
    unmasked indices first (stable), then masked ones as padding."""
    perm = np.argsort(1 - mask, kind="stable")
    take = perm[: n_chunks * P]
    rows = np.ascontiguousarray(
        data[take].reshape(n_chunks, P, -1).transpose(1, 0, 2))
    mgv = np.ascontiguousarray(mask[take].reshape(n_chunks, P).T.astype(np.int32))
    return rows, mgv


def make_in_maps(text, modality, text_mask, modality_mask,
                 text_weight, modality_weight, text_modality_weight):
    text = np.ascontiguousarray(np.asarray(text, dtype=np.float32))
    modality = np.ascontiguousarray(np.asarray(modality, dtype=np.float32))
    text_mask = np.asarray(text_mask).astype(np.int32)
    modality_mask = np.asarray(modality_mask).astype(np.int32)
    wt = np.ascontiguousarray(np.asarray(text_weight, dtype=np.float32).reshape(H, 1))
    wm = np.ascontiguousarray(
        np.asarray(modality_weight, dtype=np.float32).reshape(H, 1))
    wtm = np.ascontiguousarray(
        np.asarray(text_modality_weight, dtype=np.float32).reshape(H, 1))

    lu_counts = text_mask.sum(axis=1)
    mu_counts = modality_mask.sum(axis=1)
    LU = max(1, int(-(-int(lu_counts.max()) // P)))
    MU = max(1, int(-(-int(mu_counts.max()) // P)))

    in_maps = []
    for c in range(NCORES):
        sl = slice(BPC * c, BPC * (c + 1))
        textg = np.empty((BPC, P, LU, H), np.float32)
        modgr = np.empty((BPC, P, MU, H), np.float32)
        tmg = np.empty((BPC, P, LU), np.int32)
        mmg = np.empty((BPC, P, MU), np.int32)
        for b in range(BPC):
            gb = BPC * c + b
            textg[b], tmg[b] = _gather_meta(text_mask[gb], LU, text[gb])
            modgr[b], mmg[b] = _gather_meta(modality_mask[gb], MU, modality[gb])
        in_maps.append({
            "text": np.ascontiguousarray(text[sl]),
            "text_g": textg, "mod_g": modgr,
            "tmask_g": tmg, "mmask_g": mmg,
            "w_text": wt, "w_mod": wm, "w_tm": wtm,
        })
    return in_maps, MU, LU


def kernel(text, modality, text_mask, modality_mask,
           text_weight, modality_weight, text_modality_weight, bias,
           trace=False):
    from concourse.bass_utils import run_bass_kernel_spmd

    in_maps, MU, LU = make_in_maps(text, modality, text_mask, modality_mask,
                                   text_weight, modality_weight,
                                   text_modality_weight)
    nc = get_nc(MU, LU)
    res = run_bass_kernel_spmd(nc, in_maps, core_ids=list(range(NCORES)),
                               trace=trace)
    outp = np.concatenate([r["out"] for r in res.results], axis=0)
    if trace:
        kernel.last_result = res
    return outp


# revision 20
# speedup vs baseline: 1.1154x; 1.1154x over previous
"""BiDAF attention kernel for Trainium2 (8 NeuronCores, data-parallel over batch).

Problem (per full input): B=16, L=M=1024, H=128
  s  = text@tw + (mod@mw).T + (text*tmw)@mod.T + bias          (B, L, M)
  p1 = softmax_M(mmask*s + (1-mmask)*NEG)
  p2 = softmax_L(tmask*s + (1-tmask)*NEG)
  a  = p1 @ mod
  b  = p1 @ p2.T @ text        (computed as p1 @ (p2.T @ text))
  out = [text, a, text*a, text*b]                               (B, L, 4H)

Key facts used:
  * softmax_M is invariant to per-row (per-l) shifts: s0 & bias drop from p1.
  * softmax_L is invariant to per-column (per-m) shifts: s1 & bias drop from p2.
  * masking with {0,1} is equivalent to adding (mask-1)*30000 before exp.
  * a ones-column appended to the rhs of the p1/p2 contraction matmuls
    yields the softmax denominators for free (an extra output column).
  * fp32 matmuls run 2-pass (LOW_HIGH) on trn2 — all matmul operands are
    kept in bf16 (PSUM accumulation and softmax normalization stay fp32).
  * sparsity: masked m contribute exactly 0 to p1 (and masked l to p2), so
    the m- and l-spaces are compacted to the unmasked rows. The host
    computes permutation indices from the masks (metadata); the device
    gathers the rows via indirect DMA and computes only ceil(Mu/128) /
    ceil(Lu/128) chunks. Output rows (all l) are never compacted.

Each of the 8 cores processes 2 batch items; no cross-core communication.
"""

import numpy as np

B, L, M, H = 16, 1024, 1024, 128
NCORES = 8
BPC = B // NCORES  # batches per core
P = 128
LT, MT = L // P, M // P
NEGB = 30000.0

_CACHE = {}


def _build(MU, LU):
    """Builds the per-core Bass program for MU gathered m-chunks and LU
    gathered l-chunks (SPMD: same NEFF on all 8 cores)."""
    from contextlib import ExitStack

    import concourse.bass as bass
    import concourse.mybir as mybir
    import concourse.tile as tile
    from concourse import bacc
    from concourse.bass import ts
    from concourse.masks import make_identity

    f32 = mybir.dt.float32
    bf16 = mybir.dt.bfloat16
    i32 = mybir.dt.int32
    Exp = mybir.ActivationFunctionType.Exp
    Alu = mybir.AluOpType

    nc = bacc.Bacc(name="bidaf8")
    text = nc.dram_tensor("text", (BPC, L, H), f32, kind="ExternalInput").ap()
    # gathered-space metadata (host-computed from the masks):
    #   lidx/midx: [p, c] = flattened row index (b*L + perm[c*128+p])
    #   tmg/mmg:   [p, c] = mask value at that gathered position (0/1)
    textg = nc.dram_tensor("text_g", (BPC, P, LU, H), f32,
                           kind="ExternalInput").ap()
    modg = nc.dram_tensor("mod_g", (BPC, P, MU, H), f32,
                          kind="ExternalInput").ap()
    tmg = nc.dram_tensor("tmask_g", (BPC, P, LU), i32, kind="ExternalInput").ap()
    mmg = nc.dram_tensor("mmask_g", (BPC, P, MU), i32, kind="ExternalInput").ap()
    wt = nc.dram_tensor("w_text", (H, 1), f32, kind="ExternalInput").ap()
    wm = nc.dram_tensor("w_mod", (H, 1), f32, kind="ExternalInput").ap()
    wtm = nc.dram_tensor("w_tm", (H, 1), f32, kind="ExternalInput").ap()
    out = nc.dram_tensor("out", (BPC, L, 4 * H), f32, kind="ExternalOutput").ap()

    MG = MU * P  # gathered m columns
    NE2 = [min(512, MG - i * 512) for i in range((MG + 511) // 512)]

    def rep_rows(col_ap):
        # (H, 1) DRAM column -> broadcast AP read as (P, H): every partition
        # reads the same H contiguous floats. (gpsimd DMA only)
        return bass.AP(tensor=col_ap.tensor, offset=col_ap.offset,
                       ap=[[0, P], col_ap.ap[0]])

    with tile.TileContext(nc) as tc, ExitStack() as ctx:
        const = ctx.enter_context(tc.tile_pool(name="const", bufs=1))
        oper = ctx.enter_context(tc.tile_pool(name="oper", bufs=2))
        big = ctx.enter_context(tc.tile_pool(name="big", bufs=2))
        small = ctx.enter_context(tc.tile_pool(name="small", bufs=2))
        outp = ctx.enter_context(tc.tile_pool(name="outp", bufs=4))
        ps_s = ctx.enter_context(tc.tile_pool(name="ps_s", bufs=3, space="PSUM"))
        ps_q = ctx.enter_context(tc.tile_pool(name="ps_q", bufs=5, space="PSUM"))

        ident16 = const.tile([P, P], bf16)
        make_identity(nc, ident16)
        wtm_sb = const.tile([P, 1], f32)
        nc.sync.dma_start(wtm_sb, wtm)
        wt_rep = const.tile([P, H], f32)
        nc.gpsimd.dma_start(wt_rep, rep_rows(wt))
        wm_rep = const.tile([P, H], f32)
        nc.gpsimd.dma_start(wm_rep, rep_rows(wm))

        st = []  # per-batch tiles
        for b in range(BPC):
            d = {}
            st.append(d)
            # ---- gathered masks -> bias partials ----
            tmgi = small.tile([P, LU], i32, tag="tmgi")
            nc.gpsimd.dma_start(tmgi, tmg[b])
            d["bias2"] = small.tile([P, LU], f32, tag="bias2", name="bias2")  # per gathered l
            tmgf = small.tile([P, LU], f32, tag="tmgf")
            nc.vector.tensor_copy(tmgf, tmgi)
            nc.vector.tensor_scalar(d["bias2"], tmgf, 1.0, NEGB,
                                    op0=Alu.subtract, op1=Alu.mult)
            mmgi = small.tile([P, MU], i32, tag="mmgi")
            nc.gpsimd.dma_start(mmgi, mmg[b])
            d["bias1"] = small.tile([P, MU], f32, tag="bias1", name="bias1")  # per gathered m
            mmgf = small.tile([P, MU], f32, tag="mmgf")
            nc.vector.tensor_copy(mmgf, mmgi)
            nc.vector.tensor_scalar(d["bias1"], mmgf, 1.0, NEGB,
                                    op0=Alu.subtract, op1=Alu.mult)

            # ---- natural text load + host-gathered row loads ----
            d["txt"] = oper.tile([P, LT, H], f32, tag="txt", name="txt")
            nc.sync.dma_start(d["txt"],
                              text[b].rearrange("(p o) h -> p o h", p=P))
            txtg = oper.tile([P, LU, H], f32, tag="txtg")
            nc.scalar.dma_start(txtg, textg[b])
            modsg = oper.tile([P, MU, H], f32, tag="modsg")
            nc.sync.dma_start(modsg, modg[b])

            # ---- bf16 casts ----
            d["txt16"] = oper.tile([P, LT, H], bf16, tag="txt16", name="txt16")
            nc.vector.tensor_copy(d["txt16"], d["txt"])
            d["txtg16"] = oper.tile([P, LU, H + 1], bf16, tag="txtg16", name="txtg16")
            nc.vector.memset(d["txtg16"][:, :, H : H + 1], 1.0)
            nc.vector.tensor_copy(d["txtg16"][:, :, :H], txtg)
            d["modwq"] = big.tile([P, MU, 2 * H + 1], bf16, tag="modwq", name="modwq")
            nc.vector.memset(d["modwq"][:, :, 2 * H : 2 * H + 1], 1.0)
            nc.vector.tensor_copy(d["modwq"][:, :, :H], modsg)

            # ---- s0 (gathered l) / s1 (gathered m) row-dots on DVE ----
            s0col = small.tile([P, LU], f32, tag="s0col")
            for c in range(LU):
                scr = small.tile([P, H], f32, tag="scr")
                nc.vector.scalar_tensor_tensor(
                    out=scr, in0=txtg[:, c, :], scalar=1.0, in1=wt_rep,
                    op0=Alu.mult, op1=Alu.mult,
                    accum_out=s0col[:, c : c + 1])
            nc.vector.tensor_add(d["bias2"], d["bias2"], s0col)
            s1col = small.tile([P, MU], f32, tag="s1col")
            for c in range(MU):
                scr = small.tile([P, H], f32, tag="scr")
                nc.vector.scalar_tensor_tensor(
                    out=scr, in0=modsg[:, c, :], scalar=1.0, in1=wm_rep,
                    op0=Alu.mult, op1=Alu.mult,
                    accum_out=s1col[:, c : c + 1])
            nc.vector.tensor_add(d["bias1"], d["bias1"], s1col)

        for b in range(BPC):
            d = st[b]
            txt, txt16, txtg16 = d["txt"], d["txt16"], d["txtg16"]
            modwq, bias1, bias2 = d["modwq"], d["bias1"], d["bias2"]

            # ---- transposes (bf16) ----
            # txtT: (H, L) all l (rhs of E1T matmul); XgT: (H, LU*128)
            # gathered l, scaled by w_tm (lhsT of E2 matmul);
            # modTg: (H, MU*128) gathered m
            txtT = oper.tile([P, L], bf16, tag="txtT")
            for j in range(LT):
                tp = ps_q.tile([P, P], bf16, tag="q")
                nc.tensor.transpose(tp, txt16[:, j, :], ident16)
                nc.vector.tensor_copy(txtT[:, ts(j, P)], tp)
            XgT = oper.tile([P, LU * P], bf16, tag="XgT")
            for c in range(LU):
                tp = ps_q.tile([P, P], bf16, tag="q")
                nc.tensor.transpose(tp, txtg16[:, c, :H], ident16)
                nc.vector.tensor_copy(XgT[:, ts(c, P)], tp)
            modTg = oper.tile([P, MU * P], bf16, tag="modTg")
            for c in range(MU):
                tp = ps_q.tile([P, P], bf16, tag="q")
                nc.tensor.transpose(tp, modwq[:, c, :H], ident16)
                nc.vector.tensor_copy(modTg[:, ts(c, P)], tp)

            # scale by w_tm (per-partition h)
            nc.vector.tensor_scalar_mul(txtT, txtT, wtm_sb)
            nc.vector.tensor_scalar_mul(XgT, XgT, wtm_sb)

            # ---- E2[lg, mg] = exp(sg + bias2[lg]) ----
            E2 = big.tile([P, LU, MG], bf16, tag="E2")
            for c in range(LU):
                for hi, n in enumerate(NE2):
                    hs = slice(hi * 512, hi * 512 + n)
                    sp = ps_s.tile([P, 512], f32, tag="s")
                    nc.tensor.matmul(sp[:, :n], XgT[:, ts(c, P)], modTg[:, hs],
                                     start=True, stop=True)
                    nc.scalar.activation(E2[:, c, hs], sp[:, :n], Exp,
                                         bias=bias2[:, c : c + 1], scale=1.0)

            # ---- E1T[mg, l] = exp(sTg + bias1[mg]) interleaved with q2 ----
            E1T = big.tile([P, MU, L], bf16, tag="E1T")
            for k in range(MU):
                for half in range(2):
                    hs = ts(half, 512)
                    sp = ps_s.tile([P, 512], f32, tag="s")
                    nc.tensor.matmul(sp, modTg[:, ts(k, P)], txtT[:, hs],
                                     start=True, stop=True)
                    nc.scalar.activation(E1T[:, k, hs], sp, Exp,
                                         bias=bias1[:, k : k + 1], scale=1.0)
                # q2[mg,:] = E2.T @ [text_g|1]; wq = q2/D2
                qp = ps_q.tile([P, H + 1], f32, tag="q")
                for c in range(LU):
                    nc.tensor.matmul(qp, E2[:, c, ts(k, P)], txtg16[:, c, :],
                                     start=(c == 0), stop=(c == LU - 1))
                rec = small.tile([P, 1], f32, tag="rec2")
                nc.vector.reciprocal(rec, qp[:, H : H + 1])
                nc.vector.tensor_scalar_mul(modwq[:, k, H : 2 * H], qp[:, :H], rec)

            # ---- fused [a | D1 | b] = E1 @ [mod | 1 | wq]; assemble out ----
            for j in range(LT):
                pa = ps_q.tile([P, 2 * H + 1], f32, tag="q")
                for k in range(MU):
                    nc.tensor.matmul(pa, E1T[:, k, ts(j, P)], modwq[:, k, :],
                                     start=(k == 0), stop=(k == MU - 1))
                rec1 = small.tile([P, 1], f32, tag="rec1")
                nc.vector.reciprocal(rec1, pa[:, 2 * H : 2 * H + 1])
                o = outp.tile([P, 4 * H], f32, tag="o")
                nc.gpsimd.tensor_copy(o[:, 0:H], txt[:, j, :])
                # o[:, H:2H] = a = a_raw/D1 ; o[:, 3H:4H] = b = b_raw/D1
                ov = o[:, H:].rearrange("p (c h) -> p c h", h=H)[:, 0:3:2, :]
                pav = pa[:, : 2 * H].rearrange("p (c h) -> p c h", h=H)
                nc.vector.tensor_scalar_mul(ov, pav, rec1)
                # o[:, 2H:4H] = [text*a | text*b] in one fused op
                txtb = txt[:, j, None, :].to_broadcast((P, 2, H))
                nc.vector.scalar_tensor_tensor(
                    out=o[:, 2 * H :].rearrange("p (c h) -> p c h", h=H),
                    in0=pav, scalar=rec1, in1=txtb,
                    op0=Alu.mult, op1=Alu.mult)
                nc.sync.dma_start(
                    out[b].rearrange("(p o) c -> p o c", p=P)[:, j, :], o
                )
    nc.compile()
    return nc


def get_nc(MU, LU):
    key = (MU, LU)
    if key not in _CACHE:
        _CACHE[key] = _build(MU, LU)
    return _CACHE[key]


def _gather_meta(mask, n_chunks, data):
    """mask: (N,) 0/1 int; data: (N, H). Returns (rows, mg):
    rows (P, n_chunks, H) f32 with [p, c] = data[perm[c*128+p]] and
    mg (P, n_chunks) i32 the mask at those positions, where perm lists
    unmasked indices first (stable), then masked ones as padding."""
    perm = np.argsort(1 - mask, kind="stable")
    take = perm[: n_chunks * P]
    rows = np.ascontiguousarray(
        data[take].reshape(n_chunks, P, -1).transpose(1, 0, 2))
    mgv = np.ascontiguousarray(mask[take].reshape(n_chunks, P).T.astype(np.int32))
    return rows, mgv


def make_in_maps(text, modality, text_mask, modality_mask,
                 text_weight, modality_weight, text_modality_weight):
    text = np.ascontiguousarray(np.asarray(text, dtype=np.float32))
    modality = np.ascontiguousarray(np.asarray(modality, dtype=np.float32))
    text_mask = np.asarray(text_mask).astype(np.int32)
    modality_mask = np.asarray(modality_mask).astype(np.int32)
    wt = np.ascontiguousarray(np.asarray(text_weight, dtype=np.float32).reshape(H, 1))
    wm = np.ascontiguousarray(
        np.asarray(modality_weight, dtype=np.float32).reshape(H, 1))
    wtm = np.ascontiguousarray(
        np.asarray(text_modality_weight, dtype=np.float32).reshape(H, 1))

    lu_counts = text_mask.sum(axis=1)
    mu_counts = modality_mask.sum(axis=1)
    LU = max(1, int(-(-int(lu_counts.max()) // P)))
    MU = max(1, int(-(-int(mu_counts.max()) // P)))

    in_maps = []
    for c in range(NCORES):
        sl = slice(BPC * c, BPC * (c + 1))
        textg = np.empty((BPC, P, LU, H), np.float32)
        modgr = np.empty((BPC, P, MU, H), np.float32)
        tmg = np.empty((BPC, P, LU), np.int32)
        mmg = np.empty((BPC, P, MU), np.int32)
        for b in range(BPC):
            gb = BPC * c + b
            textg[b], tmg[b] = _gather_meta(text_mask[gb], LU, text[gb])
            modgr[b], mmg[b] = _gather_meta(modality_mask[gb], MU, modality[gb])
        in_maps.append({
            "text": np.ascontiguousarray(text[sl]),
            "text_g": textg, "mod_g": modgr,
            "tmask_g": tmg, "mmask_g": mmg,
            "w_text": wt, "w_mod": wm, "w_tm": wtm,
        })
    return in_maps, MU, LU


def kernel(text, modality, text_mask, modality_mask,
           text_weight, modality_weight, text_modality_weight, bias,
           trace=False):
    from concourse.bass_utils import run_bass_kernel_spmd

    in_maps, MU, LU = make_in_maps(text, modality, text_mask, modality_mask,
                                   text_weight, modality_weight,
                                   text_modality_weight)
    nc = get_nc(MU, LU)
    res = run_bass_kernel_spmd(nc, in_maps, core_ids=list(range(NCORES)),
                               trace=trace)
    outp = np.concatenate([r["out"] for r in res.results], axis=0)
    if trace:
        kernel.last_result = res
    return outp


# revision 21
# speedup vs baseline: 1.2900x; 1.1565x over previous
"""BiDAF attention kernel for Trainium2 (8 NeuronCores, data-parallel over batch).

Problem (per full input): B=16, L=M=1024, H=128
  s  = text@tw + (mod@mw).T + (text*tmw)@mod.T + bias          (B, L, M)
  p1 = softmax_M(mmask*s + (1-mmask)*NEG)
  p2 = softmax_L(tmask*s + (1-tmask)*NEG)
  a  = p1 @ mod
  b  = p1 @ p2.T @ text        (computed as p1 @ (p2.T @ text))
  out = [text, a, text*a, text*b]                               (B, L, 4H)

Key facts used:
  * softmax_M is invariant to per-row (per-l) shifts: s0 & bias drop from p1.
  * softmax_L is invariant to per-column (per-m) shifts: s1 & bias drop from p2.
  * masking with {0,1} is equivalent to adding (mask-1)*30000 before exp.
  * a ones-column appended to the rhs of the p1/p2 contraction matmuls
    yields the softmax denominators for free (an extra output column).
  * fp32 matmuls run 2-pass (LOW_HIGH) on trn2 — all matmul operands are
    kept in bf16 (PSUM accumulation and softmax normalization stay fp32).
  * sparsity: masked m contribute exactly 0 to p1 (and masked l to p2), so
    the m- and l-spaces are compacted to the unmasked rows. The host
    computes permutation indices from the masks (metadata); the device
    gathers the rows via indirect DMA and computes only ceil(Mu/128) /
    ceil(Lu/128) chunks. Output rows (all l) are never compacted.

Each of the 8 cores processes 2 batch items; no cross-core communication.
"""

import numpy as np

B, L, M, H = 16, 1024, 1024, 128
NCORES = 8
BPC = B // NCORES  # batches per core
P = 128
LT, MT = L // P, M // P
NEGB = 30000.0

_CACHE = {}


def _build(MU, LU):
    """Builds the per-core Bass program for MU gathered m-chunks and LU
    gathered l-chunks (SPMD: same NEFF on all 8 cores)."""
    from contextlib import ExitStack

    import concourse.bass as bass
    import concourse.mybir as mybir
    import concourse.tile as tile
    from concourse import bacc
    from concourse.bass import ts
    from concourse.masks import make_identity

    f32 = mybir.dt.float32
    bf16 = mybir.dt.bfloat16
    i32 = mybir.dt.int32
    Exp = mybir.ActivationFunctionType.Exp
    Alu = mybir.AluOpType

    nc = bacc.Bacc(name="bidaf8")
    text = nc.dram_tensor("text", (BPC, L, H), f32, kind="ExternalInput").ap()
    # gathered-space metadata (host-computed from the masks):
    #   lidx/midx: [p, c] = flattened row index (b*L + perm[c*128+p])
    #   tmg/mmg:   [p, c] = mask value at that gathered position (0/1)
    textg = nc.dram_tensor("text_g", (BPC, P, LU, H), f32,
                           kind="ExternalInput").ap()
    modg = nc.dram_tensor("mod_g", (BPC, P, MU, H), f32,
                          kind="ExternalInput").ap()
    tmg = nc.dram_tensor("tmask_g", (BPC, P, LU), i32, kind="ExternalInput").ap()
    mmg = nc.dram_tensor("mmask_g", (BPC, P, MU), i32, kind="ExternalInput").ap()
    wt = nc.dram_tensor("w_text", (H, 1), f32, kind="ExternalInput").ap()
    wm = nc.dram_tensor("w_mod", (H, 1), f32, kind="ExternalInput").ap()
    wtm = nc.dram_tensor("w_tm", (H, 1), f32, kind="ExternalInput").ap()
    out = nc.dram_tensor("out", (BPC, L, 4 * H), f32, kind="ExternalOutput").ap()

    MG = MU * P  # gathered m columns
    NE2 = [min(512, MG - i * 512) for i in range((MG + 511) // 512)]

    def rep_rows(col_ap):
        # (H, 1) DRAM column -> broadcast AP read as (P, H): every partition
        # reads the same H contiguous floats. (gpsimd DMA only)
        return bass.AP(tensor=col_ap.tensor, offset=col_ap.offset,
                       ap=[[0, P], col_ap.ap[0]])

    with tile.TileContext(nc) as tc, ExitStack() as ctx:
        const = ctx.enter_context(tc.tile_pool(name="const", bufs=1))
        oper = ctx.enter_context(tc.tile_pool(name="oper", bufs=2))
        big = ctx.enter_context(tc.tile_pool(name="big", bufs=2))
        small = ctx.enter_context(tc.tile_pool(name="small", bufs=2))
        outp = ctx.enter_context(tc.tile_pool(name="outp", bufs=4))
        ps_s = ctx.enter_context(tc.tile_pool(name="ps_s", bufs=3, space="PSUM"))
        ps_q = ctx.enter_context(tc.tile_pool(name="ps_q", bufs=5, space="PSUM"))

        ident16 = const.tile([P, P], bf16)
        make_identity(nc, ident16)
        wtm_sb = const.tile([P, 1], f32)
        nc.sync.dma_start(wtm_sb, wtm)
        wt_rep = const.tile([P, H], f32)
        nc.gpsimd.dma_start(wt_rep, rep_rows(wt))
        wm_rep = const.tile([P, H], f32)
        nc.gpsimd.dma_start(wm_rep, rep_rows(wm))

        st = []  # per-batch tiles
        for b in range(BPC):
            d = {}
            st.append(d)
            # ---- gathered masks -> bias partials ----
            tmgi = small.tile([P, LU], i32, tag="tmgi")
            nc.scalar.dma_start(tmgi, tmg[b])
            d["bias2"] = small.tile([P, LU], f32, tag="bias2", name="bias2")  # per gathered l
            tmgf = small.tile([P, LU], f32, tag="tmgf")
            nc.vector.tensor_copy(tmgf, tmgi)
            nc.vector.tensor_scalar(d["bias2"], tmgf, 1.0, NEGB,
                                    op0=Alu.subtract, op1=Alu.mult)
            mmgi = small.tile([P, MU], i32, tag="mmgi")
            nc.scalar.dma_start(mmgi, mmg[b])
            d["bias1"] = small.tile([P, MU], f32, tag="bias1", name="bias1")  # per gathered m
            mmgf = small.tile([P, MU], f32, tag="mmgf")
            nc.vector.tensor_copy(mmgf, mmgi)
            nc.vector.tensor_scalar(d["bias1"], mmgf, 1.0, NEGB,
                                    op0=Alu.subtract, op1=Alu.mult)

            # ---- natural text load + host-gathered row loads ----
            d["txt"] = oper.tile([P, LT, H], f32, tag="txt", name="txt")
            nc.sync.dma_start(d["txt"],
                              text[b].rearrange("(p o) h -> p o h", p=P))
            txtg = oper.tile([P, LU, H], f32, tag="txtg")
            nc.scalar.dma_start(txtg, textg[b])
            modsg = oper.tile([P, MU, H], f32, tag="modsg")
            nc.sync.dma_start(modsg, modg[b])

            # ---- bf16 casts ----
            d["txt16"] = oper.tile([P, LT, H], bf16, tag="txt16", name="txt16")
            nc.vector.tensor_copy(d["txt16"], d["txt"])
            d["txtg16"] = oper.tile([P, LU, H + 1], bf16, tag="txtg16", name="txtg16")
            nc.vector.memset(d["txtg16"][:, :, H : H + 1], 1.0)
            nc.vector.tensor_copy(d["txtg16"][:, :, :H], txtg)
            d["modwq"] = big.tile([P, MU, 2 * H + 1], bf16, tag="modwq", name="modwq")
            nc.vector.memset(d["modwq"][:, :, 2 * H : 2 * H + 1], 1.0)
            nc.vector.tensor_copy(d["modwq"][:, :, :H], modsg)

            # ---- s0 (gathered l) / s1 (gathered m) row-dots on DVE ----
            s0col = small.tile([P, LU], f32, tag="s0col")
            for c in range(LU):
                scr = small.tile([P, H], f32, tag="scr")
                nc.vector.scalar_tensor_tensor(
                    out=scr, in0=txtg[:, c, :], scalar=1.0, in1=wt_rep,
                    op0=Alu.mult, op1=Alu.mult,
                    accum_out=s0col[:, c : c + 1])
            nc.vector.tensor_add(d["bias2"], d["bias2"], s0col)
            s1col = small.tile([P, MU], f32, tag="s1col")
            for c in range(MU):
                scr = small.tile([P, H], f32, tag="scr")
                nc.vector.scalar_tensor_tensor(
                    out=scr, in0=modsg[:, c, :], scalar=1.0, in1=wm_rep,
                    op0=Alu.mult, op1=Alu.mult,
                    accum_out=s1col[:, c : c + 1])
            nc.vector.tensor_add(d["bias1"], d["bias1"], s1col)

        for b in range(BPC):
            d = st[b]
            txt16, txtg16, modwq = d["txt16"], d["txtg16"], d["modwq"]
            # ---- transposes (bf16) ----
            # txtT: (H, L) all l (rhs of E1T matmul); XgT: (H, LU*128)
            # gathered l, scaled by w_tm (lhsT of E2 matmul);
            # modTg: (H, MU*128) gathered m
            txtT = oper.tile([P, L], bf16, tag="txtT", name="txtT")
            for j in range(LT):
                tp = ps_q.tile([P, P], bf16, tag="q")
                nc.tensor.transpose(tp, txt16[:, j, :], ident16)
                nc.vector.tensor_copy(txtT[:, ts(j, P)], tp)
            XgT = oper.tile([P, LU * P], bf16, tag="XgT", name="XgT")
            for c in range(LU):
                tp = ps_q.tile([P, P], bf16, tag="q")
                nc.tensor.transpose(tp, txtg16[:, c, :H], ident16)
                nc.vector.tensor_copy(XgT[:, ts(c, P)], tp)
            modTg = oper.tile([P, MU * P], bf16, tag="modTg", name="modTg")
            for c in range(MU):
                tp = ps_q.tile([P, P], bf16, tag="q")
                nc.tensor.transpose(tp, modwq[:, c, :H], ident16)
                nc.vector.tensor_copy(modTg[:, ts(c, P)], tp)

            # scale by w_tm (per-partition h)
            nc.vector.tensor_scalar_mul(txtT, txtT, wtm_sb)
            nc.vector.tensor_scalar_mul(XgT, XgT, wtm_sb)
            d["txtT"], d["XgT"], d["modTg"] = txtT, XgT, modTg

        for b in range(BPC):
            d = st[b]
            XgT, modTg, bias2 = d["XgT"], d["modTg"], d["bias2"]
            # ---- E2[lg, mg] = exp(sg + bias2[lg]) ----
            E2 = big.tile([P, LU, MG], bf16, tag="E2", name="E2")
            for c in range(LU):
                for hi, n in enumerate(NE2):
                    hs = slice(hi * 512, hi * 512 + n)
                    sp = ps_s.tile([P, 512], f32, tag="s")
                    nc.tensor.matmul(sp[:, :n], XgT[:, ts(c, P)], modTg[:, hs],
                                     start=True, stop=True)
                    nc.scalar.activation(E2[:, c, hs], sp[:, :n], Exp,
                                         bias=bias2[:, c : c + 1], scale=1.0)
            d["E2"] = E2

        for b in range(BPC):
            d = st[b]
            txtT, modTg, E2 = d["txtT"], d["modTg"], d["E2"]
            txtg16, modwq, bias1 = d["txtg16"], d["modwq"], d["bias1"]
            # ---- E1T[mg, l] = exp(sTg + bias1[mg]) interleaved with q2 ----
            E1T = big.tile([P, MU, L], bf16, tag="E1T", name="E1T")
            for k in range(MU):
                for half in range(2):
                    hs = ts(half, 512)
                    sp = ps_s.tile([P, 512], f32, tag="s")
                    nc.tensor.matmul(sp, modTg[:, ts(k, P)], txtT[:, hs],
                                     start=True, stop=True)
                    nc.scalar.activation(E1T[:, k, hs], sp, Exp,
                                         bias=bias1[:, k : k + 1], scale=1.0)
                # q2[mg,:] = E2.T @ [text_g|1]; wq = q2/D2
                qp = ps_q.tile([P, H + 1], f32, tag="q")
                for c in range(LU):
                    nc.tensor.matmul(qp, E2[:, c, ts(k, P)], txtg16[:, c, :],
                                     start=(c == 0), stop=(c == LU - 1))
                rec = small.tile([P, 1], f32, tag="rec2")
                nc.vector.reciprocal(rec, qp[:, H : H + 1])
                nc.vector.tensor_scalar_mul(modwq[:, k, H : 2 * H], qp[:, :H], rec)
            d["E1T"] = E1T

        for b in range(BPC):
            d = st[b]
            txt, E1T, modwq = d["txt"], d["E1T"], d["modwq"]
            # ---- fused [a | b | D1] = E1 @ [mod | wq | 1]; assemble out ----
            for j in range(LT):
                pa = ps_q.tile([P, 2 * H + 1], f32, tag="q")
                for k in range(MU):
                    nc.tensor.matmul(pa, E1T[:, k, ts(j, P)], modwq[:, k, :],
                                     start=(k == 0), stop=(k == MU - 1))
                rec1 = small.tile([P, 1], f32, tag="rec1")
                nc.vector.reciprocal(rec1, pa[:, 2 * H : 2 * H + 1])
                o = outp.tile([P, 4 * H], f32, tag="o")
                nc.gpsimd.tensor_copy(o[:, 0:H], txt[:, j, :])
                # o[:, H:2H] = a = a_raw/D1 ; o[:, 3H:4H] = b = b_raw/D1
                ov = o[:, H:].rearrange("p (c h) -> p c h", h=H)[:, 0:3:2, :]
                pav = pa[:, : 2 * H].rearrange("p (c h) -> p c h", h=H)
                nc.vector.tensor_scalar_mul(ov, pav, rec1)
                # o[:, 2H:4H] = [text*a | text*b] in one fused op
                txtb = txt[:, j, None, :].to_broadcast((P, 2, H))
                nc.vector.scalar_tensor_tensor(
                    out=o[:, 2 * H :].rearrange("p (c h) -> p c h", h=H),
                    in0=pav, scalar=rec1, in1=txtb,
                    op0=Alu.mult, op1=Alu.mult)
                nc.sync.dma_start(
                    out[b].rearrange("(p o) c -> p o c", p=P)[:, j, :], o
                )
    nc.compile()
    return nc


def get_nc(MU, LU):
    key = (MU, LU)
    if key not in _CACHE:
        _CACHE[key] = _build(MU, LU)
    return _CACHE[key]


def _gather_meta(mask, n_chunks, data):
    """mask: (N,) 0/1 int; data: (N, H). Returns (rows, mg):
    rows (P, n_chunks, H) f32 with [p, c] = data[perm[c*128+p]] and
    mg (P, n_chunks) i32 the mask at those positions, where perm lists
    unmasked indices first (stable), then masked ones as padding."""
    perm = np.argsort(1 - mask, kind="stable")
    take = perm[: n_chunks * P]
    rows = np.ascontiguousarray(
        data[take].reshape(n_chunks, P, -1).transpose(1, 0, 2))
    mgv = np.ascontiguousarray(mask[take].reshape(n_chunks, P).T.astype(np.int32))
    return rows, mgv


def make_in_maps(text, modality, text_mask, modality_mask,
                 text_weight, modality_weight, text_modality_weight):
    text = np.ascontiguousarray(np.asarray(text, dtype=np.float32))
    modality = np.ascontiguousarray(np.asarray(modality, dtype=np.float32))
    text_mask = np.asarray(text_mask).astype(np.int32)
    modality_mask = np.asarray(modality_mask).astype(np.int32)
    wt = np.ascontiguousarray(np.asarray(text_weight, dtype=np.float32).reshape(H, 1))
    wm = np.ascontiguousarray(
        np.asarray(modality_weight, dtype=np.float32).reshape(H, 1))
    wtm = np.ascontiguousarray(
        np.asarray(text_modality_weight, dtype=np.float32).reshape(H, 1))

    lu_counts = text_mask.sum(axis=1)
    mu_counts = modality_mask.sum(axis=1)
    LU = max(1, int(-(-int(lu_counts.max()) // P)))
    MU = max(1, int(-(-int(mu_counts.max()) // P)))

    in_maps = []
    for c in range(NCORES):
        sl = slice(BPC * c, BPC * (c + 1))
        textg = np.empty((BPC, P, LU, H), np.float32)
        modgr = np.empty((BPC, P, MU, H), np.float32)
        tmg = np.empty((BPC, P, LU), np.int32)
        mmg = np.empty((BPC, P, MU), np.int32)
        for b in range(BPC):
            gb = BPC * c + b
            textg[b], tmg[b] = _gather_meta(text_mask[gb], LU, text[gb])
            modgr[b], mmg[b] = _gather_meta(modality_mask[gb], MU, modality[gb])
        in_maps.append({
            "text": np.ascontiguousarray(text[sl]),
            "text_g": textg, "mod_g": modgr,
            "tmask_g": tmg, "mmask_g": mmg,
            "w_text": wt, "w_mod": wm, "w_tm": wtm,
        })
    return in_maps, MU, LU


def kernel(text, modality, text_mask, modality_mask,
           text_weight, modality_weight, text_modality_weight, bias,
           trace=False):
    from concourse.bass_utils import run_bass_kernel_spmd

    in_maps, MU, LU = make_in_maps(text, modality, text_mask, modality_mask,
                                   text_weight, modality_weight,
                                   text_modality_weight)
    nc = get_nc(MU, LU)
    res = run_bass_kernel_spmd(nc, in_maps, core_ids=list(range(NCORES)),
                               trace=trace)
    outp = np.concatenate([r["out"] for r in res.results], axis=0)
    if trace:
        kernel.last_result = res
    return outp


# revision 22
# speedup vs baseline: 1.3588x; 1.0533x over previous
"""BiDAF attention kernel for Trainium2 (8 NeuronCores, data-parallel over batch).

Problem (per full input): B=16, L=M=1024, H=128
  s  = text@tw + (mod@mw).T + (text*tmw)@mod.T + bias          (B, L, M)
  p1 = softmax_M(mmask*s + (1-mmask)*NEG)
  p2 = softmax_L(tmask*s + (1-tmask)*NEG)
  a  = p1 @ mod
  b  = p1 @ p2.T @ text        (computed as p1 @ (p2.T @ text))
  out = [text, a, text*a, text*b]                               (B, L, 4H)

Key facts used:
  * softmax_M is invariant to per-row (per-l) shifts: s0 & bias drop from p1.
  * softmax_L is invariant to per-column (per-m) shifts: s1 & bias drop from p2.
  * masking with {0,1} is equivalent to adding (mask-1)*30000 before exp.
  * a ones-column appended to the rhs of the p1/p2 contraction matmuls
    yields the softmax denominators for free (an extra output column).
  * fp32 matmuls run 2-pass (LOW_HIGH) on trn2 — all matmul operands are
    kept in bf16 (PSUM accumulation and softmax normalization stay fp32).
  * sparsity: masked m contribute exactly 0 to p1 (and masked l to p2), so
    the m- and l-spaces are compacted to the unmasked rows. The host
    computes permutation indices from the masks (metadata); the device
    gathers the rows via indirect DMA and computes only ceil(Mu/128) /
    ceil(Lu/128) chunks. Output rows (all l) are never compacted.

Each of the 8 cores processes 2 batch items; no cross-core communication.
"""

import numpy as np

B, L, M, H = 16, 1024, 1024, 128
NCORES = 8
BPC = B // NCORES  # batches per core
P = 128
LT, MT = L // P, M // P
NEGB = 30000.0

_CACHE = {}


def _build(MU, LU):
    """Builds the per-core Bass program for MU gathered m-chunks and LU
    gathered l-chunks (SPMD: same NEFF on all 8 cores)."""
    from contextlib import ExitStack

    import concourse.bass as bass
    import concourse.mybir as mybir
    import concourse.tile as tile
    from concourse import bacc
    from concourse.bass import ts
    from concourse.masks import make_identity

    f32 = mybir.dt.float32
    bf16 = mybir.dt.bfloat16
    i32 = mybir.dt.int32
    Exp = mybir.ActivationFunctionType.Exp
    Alu = mybir.AluOpType

    nc = bacc.Bacc(name="bidaf8")
    text = nc.dram_tensor("text", (BPC, L, H), f32, kind="ExternalInput").ap()
    # gathered-space metadata (host-computed from the masks):
    #   lidx/midx: [p, c] = flattened row index (b*L + perm[c*128+p])
    #   tmg/mmg:   [p, c] = mask value at that gathered position (0/1)
    textg = nc.dram_tensor("text_g", (BPC, P, LU, H), f32,
                           kind="ExternalInput").ap()
    modg = nc.dram_tensor("mod_g", (BPC, P, MU, H), f32,
                          kind="ExternalInput").ap()
    tmg = nc.dram_tensor("tmask_g", (BPC, P, LU), i32, kind="ExternalInput").ap()
    mmg = nc.dram_tensor("mmask_g", (BPC, P, MU), i32, kind="ExternalInput").ap()
    wt = nc.dram_tensor("w_text", (H, 1), f32, kind="ExternalInput").ap()
    wm = nc.dram_tensor("w_mod", (H, 1), f32, kind="ExternalInput").ap()
    wtm = nc.dram_tensor("w_tm", (H, 1), f32, kind="ExternalInput").ap()
    out = nc.dram_tensor("out", (BPC, L, 4 * H), f32, kind="ExternalOutput").ap()

    MG = MU * P  # gathered m columns
    NE2 = [min(512, MG - i * 512) for i in range((MG + 511) // 512)]

    def rep_rows(col_ap):
        # (H, 1) DRAM column -> broadcast AP read as (P, H): every partition
        # reads the same H contiguous floats. (gpsimd DMA only)
        return bass.AP(tensor=col_ap.tensor, offset=col_ap.offset,
                       ap=[[0, P], col_ap.ap[0]])

    with tile.TileContext(nc) as tc, ExitStack() as ctx:
        const = ctx.enter_context(tc.tile_pool(name="const", bufs=1))
        oper = ctx.enter_context(tc.tile_pool(name="oper", bufs=2))
        big = ctx.enter_context(tc.tile_pool(name="big", bufs=2))
        small = ctx.enter_context(tc.tile_pool(name="small", bufs=2))
        outp = ctx.enter_context(tc.tile_pool(name="outp", bufs=4))
        ps_s = ctx.enter_context(tc.tile_pool(name="ps_s", bufs=3, space="PSUM"))
        ps_q = ctx.enter_context(tc.tile_pool(name="ps_q", bufs=5, space="PSUM"))

        ident16 = const.tile([P, P], bf16)
        make_identity(nc, ident16)
        wtm_sb = const.tile([P, 1], f32)
        nc.sync.dma_start(wtm_sb, wtm)
        wt_rep = const.tile([P, H], f32)
        nc.gpsimd.dma_start(wt_rep, rep_rows(wt))
        wm_rep = const.tile([P, H], f32)
        nc.gpsimd.dma_start(wm_rep, rep_rows(wm))

        st = []  # per-batch tiles
        for b in range(BPC):
            d = {}
            st.append(d)
            # ---- gathered masks -> bias partials ----
            tmgi = small.tile([P, LU], i32, tag="tmgi")
            nc.scalar.dma_start(tmgi, tmg[b])
            d["bias2"] = small.tile([P, LU], f32, tag="bias2", name="bias2")  # per gathered l
            tmgf = small.tile([P, LU], f32, tag="tmgf")
            nc.vector.tensor_copy(tmgf, tmgi)
            nc.vector.tensor_scalar(d["bias2"], tmgf, 1.0, NEGB,
                                    op0=Alu.subtract, op1=Alu.mult)
            mmgi = small.tile([P, MU], i32, tag="mmgi")
            nc.scalar.dma_start(mmgi, mmg[b])
            d["bias1"] = small.tile([P, MU], f32, tag="bias1", name="bias1")  # per gathered m
            mmgf = small.tile([P, MU], f32, tag="mmgf")
            nc.vector.tensor_copy(mmgf, mmgi)
            nc.vector.tensor_scalar(d["bias1"], mmgf, 1.0, NEGB,
                                    op0=Alu.subtract, op1=Alu.mult)

            # ---- host-gathered row loads first (E2 critical path) ----
            modsg = oper.tile([P, MU, H], f32, tag="modsg")
            nc.sync.dma_start(modsg, modg[b])
            txtg = oper.tile([P, LU, H], f32, tag="txtg")
            nc.scalar.dma_start(txtg, textg[b])
            d["txt"] = oper.tile([P, LT, H], f32, tag="txt", name="txt")
            nc.sync.dma_start(d["txt"],
                              text[b].rearrange("(p o) h -> p o h", p=P))

            # ---- bf16 casts ----
            d["txt16"] = oper.tile([P, LT, H], bf16, tag="txt16", name="txt16")
            nc.vector.tensor_copy(d["txt16"], d["txt"])
            d["txtg16"] = oper.tile([P, LU, H + 1], bf16, tag="txtg16", name="txtg16")
            nc.vector.memset(d["txtg16"][:, :, H : H + 1], 1.0)
            nc.vector.tensor_copy(d["txtg16"][:, :, :H], txtg)
            d["modwq"] = big.tile([P, MU, 2 * H + 1], bf16, tag="modwq", name="modwq")
            nc.vector.memset(d["modwq"][:, :, 2 * H : 2 * H + 1], 1.0)
            nc.vector.tensor_copy(d["modwq"][:, :, :H], modsg)

            # ---- s0 (gathered l) / s1 (gathered m) row-dots on DVE ----
            s0col = small.tile([P, LU], f32, tag="s0col")
            for c in range(LU):
                scr = small.tile([P, H], f32, tag="scr")
                nc.vector.scalar_tensor_tensor(
                    out=scr, in0=txtg[:, c, :], scalar=1.0, in1=wt_rep,
                    op0=Alu.mult, op1=Alu.mult,
                    accum_out=s0col[:, c : c + 1])
            nc.vector.tensor_add(d["bias2"], d["bias2"], s0col)
            s1col = small.tile([P, MU], f32, tag="s1col")
            for c in range(MU):
                scr = small.tile([P, H], f32, tag="scr")
                nc.vector.scalar_tensor_tensor(
                    out=scr, in0=modsg[:, c, :], scalar=1.0, in1=wm_rep,
                    op0=Alu.mult, op1=Alu.mult,
                    accum_out=s1col[:, c : c + 1])
            nc.vector.tensor_add(d["bias1"], d["bias1"], s1col)

        for b in range(BPC):
            d = st[b]
            txt16, txtg16, modwq = d["txt16"], d["txtg16"], d["modwq"]
            # ---- transposes (bf16), grouped 4-per-PSUM-tile ----
            # modTg: (H, MU*128) gathered m (rhs of E2, lhsT of E1T);
            # XgT: (H, LU*128) gathered l, scaled by w_tm (lhsT of E2);
            # txtT: (H, L) all l (rhs of E1T matmul), scaled by w_tm
            def transpose_into(dst, srcs):
                n = len(srcs)
                g0 = 0
                while g0 < n:
                    g1 = min(g0 + 4, n)
                    tp = ps_q.tile([P, 4, P], bf16, tag="q")
                    for i in range(g0, g1):
                        nc.tensor.transpose(tp[:, i - g0, :], srcs[i], ident16)
                    nc.vector.tensor_copy(
                        dst[:, g0 * P : g1 * P],
                        tp[:, : g1 - g0, :])
                    g0 = g1
            modTg = oper.tile([P, MU * P], bf16, tag="modTg", name="modTg")
            transpose_into(modTg, [modwq[:, c, :H] for c in range(MU)])
            XgT = oper.tile([P, LU * P], bf16, tag="XgT", name="XgT")
            transpose_into(XgT, [txtg16[:, c, :H] for c in range(LU)])
            txtT = oper.tile([P, L], bf16, tag="txtT", name="txtT")
            transpose_into(txtT, [txt16[:, j, :] for j in range(LT)])

            # scale by w_tm (per-partition h)
            nc.vector.tensor_scalar_mul(XgT, XgT, wtm_sb)
            nc.vector.tensor_scalar_mul(txtT, txtT, wtm_sb)
            d["txtT"], d["XgT"], d["modTg"] = txtT, XgT, modTg

        for b in range(BPC):
            d = st[b]
            XgT, modTg, bias2 = d["XgT"], d["modTg"], d["bias2"]
            # ---- E2[lg, mg] = exp(sg + bias2[lg]) ----
            E2 = big.tile([P, LU, MG], bf16, tag="E2", name="E2")
            for c in range(LU):
                for hi, n in enumerate(NE2):
                    hs = slice(hi * 512, hi * 512 + n)
                    sp = ps_s.tile([P, 512], f32, tag="s")
                    nc.tensor.matmul(sp[:, :n], XgT[:, ts(c, P)], modTg[:, hs],
                                     start=True, stop=True)
                    nc.scalar.activation(E2[:, c, hs], sp[:, :n], Exp,
                                         bias=bias2[:, c : c + 1], scale=1.0)
            d["E2"] = E2

        for b in range(BPC):
            d = st[b]
            txtT, modTg, E2 = d["txtT"], d["modTg"], d["E2"]
            txtg16, modwq, bias1 = d["txtg16"], d["modwq"], d["bias1"]
            # ---- E1T[mg, l] = exp(sTg + bias1[mg]) interleaved with q2 ----
            E1T = big.tile([P, MU, L], bf16, tag="E1T", name="E1T")
            for k in range(MU):
                for half in range(2):
                    hs = ts(half, 512)
                    sp = ps_s.tile([P, 512], f32, tag="s")
                    nc.tensor.matmul(sp, modTg[:, ts(k, P)], txtT[:, hs],
                                     start=True, stop=True)
                    nc.scalar.activation(E1T[:, k, hs], sp, Exp,
                                         bias=bias1[:, k : k + 1], scale=1.0)
                # q2[mg,:] = E2.T @ [text_g|1]; wq = q2/D2
                qp = ps_q.tile([P, H + 1], f32, tag="q")
                for c in range(LU):
                    nc.tensor.matmul(qp, E2[:, c, ts(k, P)], txtg16[:, c, :],
                                     start=(c == 0), stop=(c == LU - 1))
                rec = small.tile([P, 1], f32, tag="rec2")
                nc.vector.reciprocal(rec, qp[:, H : H + 1])
                nc.vector.tensor_scalar_mul(modwq[:, k, H : 2 * H], qp[:, :H], rec)
            d["E1T"] = E1T

        for b in range(BPC):
            d = st[b]
            txt, E1T, modwq = d["txt"], d["E1T"], d["modwq"]
            # ---- fused [a | b | D1] = E1 @ [mod | wq | 1]; assemble out ----
            for j in range(LT):
                pa = ps_q.tile([P, 2 * H + 1], f32, tag="q")
                for k in range(MU):
                    nc.tensor.matmul(pa, E1T[:, k, ts(j, P)], modwq[:, k, :],
                                     start=(k == 0), stop=(k == MU - 1))
                rec1 = small.tile([P, 1], f32, tag="rec1")
                nc.vector.reciprocal(rec1, pa[:, 2 * H : 2 * H + 1])
                o = outp.tile([P, 4 * H], f32, tag="o")
                nc.gpsimd.tensor_copy(o[:, 0:H], txt[:, j, :])
                # o[:, H:2H] = a = a_raw/D1 ; o[:, 3H:4H] = b = b_raw/D1
                ov = o[:, H:].rearrange("p (c h) -> p c h", h=H)[:, 0:3:2, :]
                pav = pa[:, : 2 * H].rearrange("p (c h) -> p c h", h=H)
                nc.vector.tensor_scalar_mul(ov, pav, rec1)
                # o[:, 2H:4H] = [text*a | text*b] in one fused op
                txtb = txt[:, j, None, :].to_broadcast((P, 2, H))
                nc.vector.scalar_tensor_tensor(
                    out=o[:, 2 * H :].rearrange("p (c h) -> p c h", h=H),
                    in0=pav, scalar=rec1, in1=txtb,
                    op0=Alu.mult, op1=Alu.mult)
                nc.sync.dma_start(
                    out[b].rearrange("(p o) c -> p o c", p=P)[:, j, :], o
                )
    nc.compile()
    return nc


def get_nc(MU, LU):
    key = (MU, LU)
    if key not in _CACHE:
        _CACHE[key] = _build(MU, LU)
    return _CACHE[key]


def _gather_meta(mask, n_chunks, data):
    """mask: (N,) 0/1 int; data: (N, H). Returns (rows, mg):
    rows (P, n_chunks, H) f32 with [p, c] = data[perm[c*128+p]] and
    mg (P, n_chunks) i32 the mask at those positions, where perm lists
    unmasked indices first (stable), then masked ones as padding."""
    perm = np.argsort(1 - mask, kind="stable")
    take = perm[: n_chunks * P]
    rows = np.ascontiguousarray(
        data[take].reshape(n_chunks, P, -1).transpose(1, 0, 2))
    mgv = np.ascontiguousarray(mask[take].reshape(n_chunks, P).T.astype(np.int32))
    return rows, mgv


def make_in_maps(text, modality, text_mask, modality_mask,
                 text_weight, modality_weight, text_modality_weight):
    text = np.ascontiguousarray(np.asarray(text, dtype=np.float32))
    modality = np.ascontiguousarray(np.asarray(modality, dtype=np.float32))
    text_mask = np.asarray(text_mask).astype(np.int32)
    modality_mask = np.asarray(modality_mask).astype(np.int32)
    wt = np.ascontiguousarray(np.asarray(text_weight, dtype=np.float32).reshape(H, 1))
    wm = np.ascontiguousarray(
        np.asarray(modality_weight, dtype=np.float32).reshape(H, 1))
    wtm = np.ascontiguousarray(
        np.asarray(text_modality_weight, dtype=np.float32).reshape(H, 1))

    lu_counts = text_mask.sum(axis=1)
    mu_counts = modality_mask.sum(axis=1)
    LU = max(1, int(-(-int(lu_counts.max()) // P)))
    MU = max(1, int(-(-int(mu_counts.max()) // P)))

    in_maps = []
    for c in range(NCORES):
        sl = slice(BPC * c, BPC * (c + 1))
        textg = np.empty((BPC, P, LU, H), np.float32)
        modgr = np.empty((BPC, P, MU, H), np.float32)
        tmg = np.empty((BPC, P, LU), np.int32)
        mmg = np.empty((BPC, P, MU), np.int32)
        for b in range(BPC):
            gb = BPC * c + b
            textg[b], tmg[b] = _gather_meta(text_mask[gb], LU, text[gb])
            modgr[b], mmg[b] = _gather_meta(modality_mask[gb], MU, modality[gb])
        in_maps.append({
            "text": np.ascontiguousarray(text[sl]),
            "text_g": textg, "mod_g": modgr,
            "tmask_g": tmg, "mmask_g": mmg,
            "w_text": wt, "w_mod": wm, "w_tm": wtm,
        })
    return in_maps, MU, LU


def kernel(text, modality, text_mask, modality_mask,
           text_weight, modality_weight, text_modality_weight, bias,
           trace=False):
    from concourse.bass_utils import run_bass_kernel_spmd

    in_maps, MU, LU = make_in_maps(text, modality, text_mask, modality_mask,
                                   text_weight, modality_weight,
                                   text_modality_weight)
    nc = get_nc(MU, LU)
    res = run_bass_kernel_spmd(nc, in_maps, core_ids=list(range(NCORES)),
                               trace=trace)
    outp = np.concatenate([r["out"] for r in res.results], axis=0)
    if trace:
        kernel.last_result = res
    return outp
